# revision 15
# baseline (speedup 1.0000x reference)
"""KKT loss kernel v5 for Trainium2 (Bass/Tile), 8 NeuronCores.

Host sorts each problem's COO per side, packs complete scatter-rows into
2048-slot windows (slot 0 pad, span <= 127). Device per window: ap_gather
x[cols]/lam[rows] in sorted order (Q7 core j = problem j, row 16j),
s = v*g (in place), window-local inclusive prefix scan (DVE), boundary
ap_gather of the prefix at host-known row-end slots, adjacent diff =
per-row segment sums, loss terms accumulated against host-permuted
b/lam/c in row layout. No matmuls, no one-hots, no relayouts; emission is
stage-major with one-window lag on the boundary stage for pipeline slack.
"""

import os
import sys
import time

import numpy as np

sys.path.insert(0, "/opt/trn_rl_repo")

from contextlib import ExitStack

import concourse.bass as bass
import concourse.mybir as mybir
from concourse import bacc, tile

B, M, N, NNZ = 64, 8192, 8192, 262144
W_PRIMAL, W_DUAL, W_STAT, W_COMP = 0.1, 0.1, 0.6, 0.2

PB = 8
NCORES = 8
WIN = 2048
PW = 127

f32 = mybir.dt.float32
i16 = mybir.dt.int16

LAST_EXEC_NS = None
LAST_SINGLE_NS = None
_CACHED = {}


def _run_timed(nc, in_maps, n_cores, reps=5):
    """Execute the compiled Bass module on n_cores via PJRT; time repeats."""
    import time as _time

    import jax

    from jax.sharding import Mesh, PartitionSpec
    from jax.experimental.shard_map import shard_map

    from concourse import bass2jax, mybir as _mybir
    from concourse.bass2jax import _bass_exec_p, partition_id_tensor

    bass2jax.install_neuronx_cc_hook()

    partition_name = nc.partition_id_tensor.name if nc.partition_id_tensor else None
    in_names, out_names, out_avals, zero_outs = [], [], [], []
    for alloc in nc.m.functions[0].allocations:
        if not isinstance(alloc, _mybir.MemoryLocationSet):
            continue
        name = alloc.memorylocations[0].name
        if alloc.kind == "ExternalInput":
            if name != partition_name:
                in_names.append(name)
        elif alloc.kind == "ExternalOutput":
            shape = tuple(alloc.tensor_shape)
            dtype = _mybir.dt.np(alloc.dtype)
            out_names.append(name)
            out_avals.append(jax.core.ShapedArray(shape, dtype))
            zero_outs.append(np.zeros(shape, dtype))
    n_params = len(in_names)
    all_in_names = list(in_names) + list(out_names)
    if partition_name is not None:
        all_in_names.append(partition_name)

    def _body(*args):
        operands = list(args)
        if partition_name is not None:
            operands.append(partition_id_tensor())
        return tuple(
            _bass_exec_p.bind(
                *operands,
                out_avals=tuple(out_avals),
                in_names=tuple(all_in_names),
                out_names=tuple(out_names),
                lowering_input_output_aliases=(),
                sim_require_finite=False,
                sim_require_nnan=False,
                nc=nc,
            )
        )

    devices = jax.devices()[:n_cores]
    mesh = Mesh(np.asarray(devices), ("core",))
    n_outs = len(out_names)
    in_specs = (PartitionSpec("core"),) * (n_params + n_outs)
    out_specs = (PartitionSpec("core"),) * n_outs
    f1 = jax.jit(
        shard_map(_body, mesh=mesh, in_specs=in_specs, out_specs=out_specs,
                  check_rep=False),
        keep_unused=True,
    )

    if nc.dbg_addr is not None:
        dbg_zero = np.zeros((1, 2), np.uint32)
        in_maps = [{**m, nc.dbg_addr.name: dbg_zero} for m in in_maps]
    per_core = [[np.asarray(m[nm]) for nm in in_names] for m in in_maps]
    concat_in = [
        np.concatenate([per_core[c][i] for c in range(n_cores)], axis=0)
        for i in range(n_params)
    ]
    concat_zeros = [
        np.zeros((n_cores * z.shape[0], *z.shape[1:]), z.dtype) for z in zero_outs
    ]
    dev_in = [jax.device_put(a) for a in concat_in]
    dev_zeros = [jax.device_put(z) for z in concat_zeros]

    out1 = f1(*dev_in, *dev_zeros)
    jax.block_until_ready(out1)

    def _batch(k):
        best = None
        for _ in range(3):
            t0 = _time.perf_counter()
            os_ = [f1(*dev_in, *dev_zeros) for _ in range(k)]
            jax.block_until_ready(os_)
            dt = _time.perf_counter() - t0
            best = dt if best is None else min(best, dt)
        return best

    K = int(os.environ.get("KKT_TIME_K", "8"))
    tK = _batch(K)
    t2K = _batch(2 * K)
    per_exec_ns = (t2K - tK) / K * 1e9
    single_ns = tK / K * 1e9

    results = [
        {
            name: np.asarray(out1[i]).reshape(n_cores, *out_avals[i].shape)[c]
            for i, name in enumerate(out_names)
        }
        for c in range(n_cores)
    ]
    return results, per_exec_ns, single_ns




def _rows16(ap):
    """[128, F] AP -> [8, 1, F] view selecting partitions 16j."""
    return ap.rearrange("(a b) f -> a b f", b=16)[:, 0:1, :]


def build_kernel(nwv):
    G = 4
    GW = G * WIN          # 8192 slots per group
    GB = G * 128          # 512 boundary entries per group
    nc = bacc.Bacc(None, target_bir_lowering=False, debug=True)

    xs = nc.dram_tensor("xs", [PB, N], f32, kind="ExternalInput")
    lams = nc.dram_tensor("lams", [PB, M], f32, kind="ExternalInput")
    valr = nc.dram_tensor("valr", [nwv, PB, WIN], f32, kind="ExternalInput")
    valc = nc.dram_tensor("valc", [nwv, PB, WIN], f32, kind="ExternalInput")
    colsw = nc.dram_tensor("colsw", [nwv, PB, 16, 128], i16, kind="ExternalInput")
    rowsw = nc.dram_tensor("rowsw", [nwv, PB, 16, 128], i16, kind="ExternalInput")
    bndr = nc.dram_tensor("bndr", [nwv, PB, 16, 8], i16, kind="ExternalInput")
    bndc = nc.dram_tensor("bndc", [nwv, PB, 16, 8], i16, kind="ExternalInput")
    brow = nc.dram_tensor("brow", [nwv, PB, 128], f32, kind="ExternalInput")
    crow = nc.dram_tensor("crow", [nwv, PB, 128], f32, kind="ExternalInput")
    lamrow = nc.dram_tensor("lamrow", [nwv, PB, 128], f32, kind="ExternalInput")
    out = nc.dram_tensor("out", [4, PB], f32, kind="ExternalOutput")

    def rows16(ap, off=0):
        return ap.rearrange("(a b) f -> a b f", b=16)[:, off : off + 1, :]

    with tile.TileContext(nc) as tc, ExitStack() as ctx:
        const = ctx.enter_context(tc.tile_pool(name="const", bufs=1))
        zp = ctx.enter_context(tc.tile_pool(name="zp", bufs=1))
        gp = ctx.enter_context(tc.tile_pool(name="gp", bufs=1))
        st = ctx.enter_context(tc.tile_pool(name="st", bufs=2))
        pg = ctx.enter_context(tc.tile_pool(name="pg", bufs=2))
        wp = ctx.enter_context(tc.tile_pool(name="wp", bufs=1))

        # merged gather source: x_j on partition 16j, lam_j on 16j+8
        srcv = const.tile([128, N], f32, tag="srcv")
        nc.vector.memset(srcv[:], 0.0)
        for j in range(PB):
            nc.sync.dma_start(srcv[16 * j : 16 * j + 1, :], xs[j : j + 1, :])
            nc.sync.dma_start(
                srcv[16 * j + 8 : 16 * j + 9, :], lams[j : j + 1, :]
            )

        accP = const.tile([128, 1], f32, tag="accP")
        accC = const.tile([128, 1], f32, tag="accC")
        accS = const.tile([128, 1], f32, tag="accS")
        nc.vector.memset(accP[:], 0.0)
        nc.vector.memset(accC[:], 0.0)
        nc.vector.memset(accS[:], 0.0)

        # pre-zero rotating buffers whose unused rows must stay 0
        for tg in ("vr", "vc"):
            tz = zp.tile([128, GW], f32, tag=tg)
            nc.vector.memset(tz[:], 0.0)
        for tg in ("br", "cr", "lr"):
            tz = zp.tile([128, GB], f32, tag=tg)
            nc.vector.memset(tz[:], 0.0)
        for tg in ("dx", "dc"):
            tz = wp.tile([128, GB], f32, tag=tg)
            nc.vector.memset(tz[:], 0.0)

        for g in range(nwv // G):
            sl = slice(G * g, G * (g + 1))
            vr = zp.tile([128, GW], f32, tag="vr")
            nc.sync.dma_start(rows16(vr[:]), valr[sl].rearrange("w j f -> j w f"))
            vc = zp.tile([128, GW], f32, tag="vc")
            nc.sync.dma_start(
                rows16(vc[:], 8), valc[sl].rearrange("w j f -> j w f")
            )
            ixc = st.tile([128, G * 128], i16, tag="ixc")
            nc.sync.dma_start(
                ixc[:].rearrange("p (w s) -> p w s", w=G),
                colsw[sl].rearrange("w j q s -> (j q) w s"),
            )
            ixr = st.tile([128, G * 128], i16, tag="ixr")
            nc.sync.dma_start(
                ixr[:].rearrange("p (w s) -> p w s", w=G),
                rowsw[sl].rearrange("w j q s -> (j q) w s"),
            )
            ibr = st.tile([128, G * 8], i16, tag="ibr")
            nc.sync.dma_start(
                ibr[:].rearrange("p (w s) -> p w s", w=G),
                bndr[sl].rearrange("w j q s -> (j q) w s"),
            )
            ibc = st.tile([128, G * 8], i16, tag="ibc")
            nc.sync.dma_start(
                ibc[:].rearrange("p (w s) -> p w s", w=G),
                bndc[sl].rearrange("w j q s -> (j q) w s"),
            )
            br_t = zp.tile([128, GB], f32, tag="br")
            nc.sync.dma_start(rows16(br_t[:]), brow[sl].rearrange("w j f -> j w f"))
            cr_t = zp.tile([128, GB], f32, tag="cr")
            nc.sync.dma_start(
                rows16(cr_t[:], 8), crow[sl].rearrange("w j f -> j w f")
            )
            lr_t = zp.tile([128, GB], f32, tag="lr")
            nc.sync.dma_start(
                rows16(lr_t[:]), lamrow[sl].rearrange("w j f -> j w f")
            )

            gx = gp.tile([128, GW], f32, tag="gx")
            nc.gpsimd.ap_gather(gx[:], srcv[:], ixc[:], 128, N, 1, GW)
            gl = gp.tile([128, GW], f32, tag="gl")
            nc.gpsimd.ap_gather(gl[:], srcv[:], ixr[:], 128, M, 1, GW)

            nc.vector.tensor_tensor(gx[:], vr[:], gx[:], mybir.AluOpType.mult)
            nc.vector.tensor_tensor_scan(
                gx[:], gx[:], gx[:], 0.0, mybir.AluOpType.add,
                mybir.AluOpType.bypass,
            )
            nc.vector.tensor_tensor(gl[:], vc[:], gl[:], mybir.AluOpType.mult)
            nc.vector.tensor_tensor_scan(
                gl[:], gl[:], gl[:], 0.0, mybir.AluOpType.add,
                mybir.AluOpType.bypass,
            )

            Pgx = pg.tile([128, GB], f32, tag="pgx")
            nc.gpsimd.ap_gather(Pgx[:], gx[:], ibr[:], 128, GW, 1, GB)
            Pgl = pg.tile([128, GB], f32, tag="pgl")
            nc.gpsimd.ap_gather(Pgl[:], gl[:], ibc[:], 128, GW, 1, GB)

            dx = wp.tile([128, GB], f32, tag="dx")
            nc.vector.tensor_tensor(
                dx[:, 0 : GB - 1], Pgx[:, 1:GB], Pgx[:, 0 : GB - 1],
                mybir.AluOpType.subtract,
            )
            dab = wp.tile([128, GB], f32, tag="dab")
            nc.vector.tensor_tensor(dab[:], dx[:], br_t[:], mybir.AluOpType.subtract)
            rd = wp.tile([128, GB], f32, tag="rd")
            nc.vector.tensor_scalar(rd[:], dab[:], 0.0, None, mybir.AluOpType.max)
            rd2 = wp.tile([128, GB], f32, tag="rd2")
            nc.vector.tensor_tensor(rd2[:], rd[:], rd[:], mybir.AluOpType.mult)
            tpr = wp.tile([128, 1], f32, tag="tpr")
            nc.vector.tensor_reduce(
                tpr[:], rd2[:], mybir.AxisListType.X, mybir.AluOpType.add
            )
            nc.vector.tensor_tensor(accP[:], accP[:], tpr[:], mybir.AluOpType.add)
            ld = wp.tile([128, GB], f32, tag="ld")
            nc.vector.tensor_tensor(ld[:], lr_t[:], dab[:], mybir.AluOpType.mult)
            ld2 = wp.tile([128, GB], f32, tag="ld2")
            nc.vector.tensor_tensor(ld2[:], ld[:], ld[:], mybir.AluOpType.mult)
            tpc = wp.tile([128, 1], f32, tag="tpc")
            nc.vector.tensor_reduce(
                tpc[:], ld2[:], mybir.AxisListType.X, mybir.AluOpType.add
            )
            nc.vector.tensor_tensor(accC[:], accC[:], tpc[:], mybir.AluOpType.add)

            dc = wp.tile([128, GB], f32, tag="dc")
            nc.vector.tensor_tensor(
                dc[:, 0 : GB - 1], Pgl[:, 1:GB], Pgl[:, 0 : GB - 1],
                mybir.AluOpType.subtract,
            )
            stc = wp.tile([128, GB], f32, tag="stc")
            nc.vector.tensor_tensor(stc[:], dc[:], cr_t[:], mybir.AluOpType.add)
            st2 = wp.tile([128, GB], f32, tag="st2")
            nc.vector.tensor_tensor(st2[:], stc[:], stc[:], mybir.AluOpType.mult)
            tps = wp.tile([128, 1], f32, tag="tps")
            nc.vector.tensor_reduce(
                tps[:], st2[:], mybir.AxisListType.X, mybir.AluOpType.add
            )
            nc.vector.tensor_tensor(accS[:], accS[:], tps[:], mybir.AluOpType.add)

        # dual from lam rows {16j+8} of srcv (in place; gathers all done)
        nc.vector.tensor_scalar(srcv[:], srcv[:], 0.0, None, mybir.AluOpType.min)
        nc.vector.tensor_tensor(srcv[:], srcv[:], srcv[:], mybir.AluOpType.mult)
        accD = const.tile([128, 1], f32, tag="accD")
        nc.vector.tensor_reduce(
            accD[:], srcv[:], mybir.AxisListType.X, mybir.AluOpType.add
        )

        for k, (acc, off) in enumerate(
            ((accP, 0), (accC, 0), (accS, 8), (accD, 8))
        ):
            nc.sync.dma_start(out[k : k + 1, :], rows16(acc[:], off))

    nc.compile()
    return nc


def _pack_side(key_idx, other_idx, vals):
    order = np.argsort(key_idx, kind="stable")
    k_s = key_idx[order]
    o_s = other_idx[order].astype(np.int16)
    v_s = vals[order]
    counts = np.bincount(k_s, minlength=M)
    row_start = np.concatenate([[0], np.cumsum(counts)])
    spans = []
    base = 0
    while base < M:
        span = 0
        slots = 0
        while base + span < M and span < PW:
            c = counts[base + span]
            if slots + c > WIN - 1:
                break
            slots += c
            span += 1
        assert span > 0, "single row exceeds WIN-1"
        spans.append((base, span, slots))
        base += span
    return o_s, v_s, counts, row_start, spans


def _fill_side(nwv, packs):
    """Per-core side arrays: valw [nwv,PB,WIN] f32, gidx [nwv,PB,16,128] i16,
    bnd [nwv,PB,16,8] i16, prm [nwv,128,PB] int64 (-1 pad)."""
    valw = np.zeros((nwv, PB, WIN), np.float32)
    gidx = np.zeros((nwv, PB, 16, 128), np.int16)
    bnd = np.zeros((nwv, PB, 16, 8), np.int16)
    prm = np.full((nwv, 128, PB), -1, np.int64)
    for j, (o_s, v_s, counts, row_start, spans) in enumerate(packs):
        for w, (b0, span, slots) in enumerate(spans):
            lo, hi = row_start[b0], row_start[b0 + span]
            vwin = np.zeros(WIN, np.float32)
            owin = np.zeros(WIN, np.int16)
            vwin[1 : 1 + slots] = v_s[lo:hi]
            owin[1 : 1 + slots] = o_s[lo:hi]
            valw[w, j] = vwin
            gidx[w, j] = owin.reshape(128, 16).T
            cum = np.cumsum(counts[b0 : b0 + span])
            off = 2048 * (w % 4)
            bs = np.full(128, off, np.int64)
            for r in range(127):
                bs[r + 1] = off + cum[min(r, span - 1)]
            bnd[w, j] = bs.reshape(8, 16).T
            prm[w, :span, j] = np.arange(b0, b0 + span)
        for w in range(len(spans), nwv):
            bs = np.full(128, 2048 * (w % 4), np.int64)
            bnd[w, j] = bs.reshape(8, 16).T
    return valw, gidx, bnd, prm


def _apply_prm(vec, prm):
    """vec [PB, M] -> [nwv, PB, 128] permuted row-layout (0 where pad)."""
    nwv = prm.shape[0]
    outp = np.zeros((nwv, PB, 128), np.float32)
    for j in range(PB):
        m = prm[:, :, j]
        valid = m >= 0
        outp[:, j, :][valid] = vec[j][m[valid]]
    return outp


def kernel(x_hat, lam_hat, A_vals, A_rows, A_cols, b_pad, c_pad):
    global LAST_EXEC_NS, LAST_SINGLE_NS
    x = np.asarray(x_hat, dtype=np.float32).reshape(B, N)
    lam = np.asarray(lam_hat, dtype=np.float32).reshape(B, M)
    A_vals = np.ascontiguousarray(np.asarray(A_vals, dtype=np.float32))
    A_rows = np.ascontiguousarray(np.asarray(A_rows, dtype=np.int32))
    A_cols = np.ascontiguousarray(np.asarray(A_cols, dtype=np.int32))
    b_pad = np.ascontiguousarray(np.asarray(b_pad, dtype=np.float32))
    c_pad = np.ascontiguousarray(np.asarray(c_pad, dtype=np.float32))

    try:
        t0 = time.time()
        packs_r = [[None] * PB for _ in range(NCORES)]
        packs_c = [[None] * PB for _ in range(NCORES)]
        nwv = 0
        for i in range(B):
            ci, j = divmod(i, PB)
            pr = _pack_side(A_rows[i], A_cols[i], A_vals[i])
            pc = _pack_side(A_cols[i], A_rows[i], A_vals[i])
            packs_r[ci][j] = pr
            packs_c[ci][j] = pc
            nwv = max(nwv, len(pr[4]), len(pc[4]))
        nwv = (nwv + 3) // 4 * 4
        in_maps = []
        for ci in range(NCORES):
            s = slice(PB * ci, PB * (ci + 1))
            valr, colsw, bndr, prm_r = _fill_side(nwv, packs_r[ci])
            valc, rowsw, bndc, prm_c = _fill_side(nwv, packs_c[ci])
            in_maps.append(
                {
                    "xs": np.ascontiguousarray(x[s]),
                    "lams": np.ascontiguousarray(lam[s]),
                    "valr": valr,
                    "valc": valc,
                    "colsw": colsw,
                    "rowsw": rowsw,
                    "bndr": bndr,
                    "bndc": bndc,
                    "brow": _apply_prm(b_pad[s], prm_r),
                    "lamrow": _apply_prm(lam[s], prm_r),
                    "crow": _apply_prm(c_pad[s], prm_c),
                }
            )
        print(f"[kernel] host prep {time.time()-t0:.1f}s, nwv={nwv}", flush=True)

        key = ("nc", nwv)
        if key not in _CACHED:
            _CACHED[key] = build_kernel(nwv)
        nc = _CACHED[key]

        results, per_exec_ns, single_ns = _run_timed(nc, in_maps, NCORES)
        LAST_EXEC_NS = per_exec_ns
        LAST_SINGLE_NS = single_ns
        print(
            f"[kernel] per-exec {per_exec_ns:.0f} ns, best {single_ns:.0f} ns",
            flush=True,
        )
    except Exception:
        import traceback

        traceback.print_exc()
        return _host_fallback(x, lam, A_vals, A_rows, A_cols, b_pad, c_pad)

    total = np.float64(0.0)
    for i in range(NCORES):
        v = np.asarray(results[i]["out"], dtype=np.float64).reshape(4, PB)
        for j in range(PB):
            total += (
                W_PRIMAL * v[0, j] / M
                + W_COMP * v[1, j] / M
                + W_STAT * v[2, j] / N
                + W_DUAL * v[3, j] / M
            )
    return np.float32(total / B)


def _host_fallback(x, lam, vals, rows, cols, b_pad, c_pad):
    print("[kernel] !!! HOST FALLBACK USED !!!", flush=True)
    tot = 0.0
    for i in range(B):
        Ax = np.bincount(rows[i], weights=(vals[i] * x[i][cols[i]]).astype(np.float64), minlength=M)
        ATl = np.bincount(cols[i], weights=(vals[i] * lam[i][rows[i]]).astype(np.float64), minlength=N)
        d = Ax - b_pad[i]
        tot += (W_PRIMAL * np.mean(np.maximum(d, 0.0) ** 2)
                + W_DUAL * np.mean(np.maximum(-lam[i], 0.0) ** 2)
                + W_STAT * np.mean((ATl + c_pad[i]) ** 2)
                + W_COMP * np.mean((lam[i] * d) ** 2))
    return np.float32(tot / B)


# revision 16
# speedup vs baseline: 1.0089x; 1.0089x over previous
"""KKT loss kernel v5 for Trainium2 (Bass/Tile), 8 NeuronCores.

Host sorts each problem's COO per side, packs complete scatter-rows into
2048-slot windows (slot 0 pad, span <= 127). Device per window: ap_gather
x[cols]/lam[rows] in sorted order (Q7 core j = problem j, row 16j),
s = v*g (in place), window-local inclusive prefix scan (DVE), boundary
ap_gather of the prefix at host-known row-end slots, adjacent diff =
per-row segment sums, loss terms accumulated against host-permuted
b/lam/c in row layout. No matmuls, no one-hots, no relayouts; emission is
stage-major with one-window lag on the boundary stage for pipeline slack.
"""

import os
import sys
import time

import numpy as np

sys.path.insert(0, "/opt/trn_rl_repo")

from contextlib import ExitStack

import concourse.bass as bass
import concourse.mybir as mybir
from concourse import bacc, tile

B, M, N, NNZ = 64, 8192, 8192, 262144
W_PRIMAL, W_DUAL, W_STAT, W_COMP = 0.1, 0.1, 0.6, 0.2

PB = 8
NCORES = 8
WIN = 2048
PW = 127

f32 = mybir.dt.float32
bf16 = mybir.dt.bfloat16
i16 = mybir.dt.int16

LAST_EXEC_NS = None
LAST_SINGLE_NS = None
_CACHED = {}


def _run_timed(nc, in_maps, n_cores, reps=5):
    """Execute the compiled Bass module on n_cores via PJRT; time repeats."""
    import time as _time

    import jax

    from jax.sharding import Mesh, PartitionSpec
    from jax.experimental.shard_map import shard_map

    from concourse import bass2jax, mybir as _mybir
    from concourse.bass2jax import _bass_exec_p, partition_id_tensor

    bass2jax.install_neuronx_cc_hook()

    partition_name = nc.partition_id_tensor.name if nc.partition_id_tensor else None
    in_names, out_names, out_avals, zero_outs = [], [], [], []
    for alloc in nc.m.functions[0].allocations:
        if not isinstance(alloc, _mybir.MemoryLocationSet):
            continue
        name = alloc.memorylocations[0].name
        if alloc.kind == "ExternalInput":
            if name != partition_name:
                in_names.append(name)
        elif alloc.kind == "ExternalOutput":
            shape = tuple(alloc.tensor_shape)
            dtype = _mybir.dt.np(alloc.dtype)
            out_names.append(name)
            out_avals.append(jax.core.ShapedArray(shape, dtype))
            zero_outs.append(np.zeros(shape, dtype))
    n_params = len(in_names)
    all_in_names = list(in_names) + list(out_names)
    if partition_name is not None:
        all_in_names.append(partition_name)

    def _body(*args):
        operands = list(args)
        if partition_name is not None:
            operands.append(partition_id_tensor())
        return tuple(
            _bass_exec_p.bind(
                *operands,
                out_avals=tuple(out_avals),
                in_names=tuple(all_in_names),
                out_names=tuple(out_names),
                lowering_input_output_aliases=(),
                sim_require_finite=False,
                sim_require_nnan=False,
                nc=nc,
            )
        )

    devices = jax.devices()[:n_cores]
    mesh = Mesh(np.asarray(devices), ("core",))
    n_outs = len(out_names)
    in_specs = (PartitionSpec("core"),) * (n_params + n_outs)
    out_specs = (PartitionSpec("core"),) * n_outs
    f1 = jax.jit(
        shard_map(_body, mesh=mesh, in_specs=in_specs, out_specs=out_specs,
                  check_rep=False),
        keep_unused=True,
    )

    if nc.dbg_addr is not None:
        dbg_zero = np.zeros((1, 2), np.uint32)
        in_maps = [{**m, nc.dbg_addr.name: dbg_zero} for m in in_maps]
    per_core = [[np.asarray(m[nm]) for nm in in_names] for m in in_maps]
    concat_in = [
        np.concatenate([per_core[c][i] for c in range(n_cores)], axis=0)
        for i in range(n_params)
    ]
    concat_zeros = [
        np.zeros((n_cores * z.shape[0], *z.shape[1:]), z.dtype) for z in zero_outs
    ]
    dev_in = [jax.device_put(a) for a in concat_in]
    dev_zeros = [jax.device_put(z) for z in concat_zeros]

    out1 = f1(*dev_in, *dev_zeros)
    jax.block_until_ready(out1)

    def _batch(k):
        best = None
        for _ in range(3):
            t0 = _time.perf_counter()
            os_ = [f1(*dev_in, *dev_zeros) for _ in range(k)]
            jax.block_until_ready(os_)
            dt = _time.perf_counter() - t0
            best = dt if best is None else min(best, dt)
        return best

    K = int(os.environ.get("KKT_TIME_K", "8"))
    tK = _batch(K)
    t2K = _batch(2 * K)
    per_exec_ns = (t2K - tK) / K * 1e9
    single_ns = tK / K * 1e9

    results = [
        {
            name: np.asarray(out1[i]).reshape(n_cores, *out_avals[i].shape)[c]
            for i, name in enumerate(out_names)
        }
        for c in range(n_cores)
    ]
    return results, per_exec_ns, single_ns




def _rows16(ap):
    """[128, F] AP -> [8, 1, F] view selecting partitions 16j."""
    return ap.rearrange("(a b) f -> a b f", b=16)[:, 0:1, :]


def build_kernel(nwv):
    G = 4
    GW = G * WIN          # 8192 slots per group
    GB = G * 128          # 512 boundary entries per group
    nc = bacc.Bacc(None, target_bir_lowering=False, debug=True)

    xs = nc.dram_tensor("xs", [PB, N], f32, kind="ExternalInput")
    lams = nc.dram_tensor("lams", [PB, M], f32, kind="ExternalInput")
    valr = nc.dram_tensor("valr", [nwv, PB, WIN], bf16, kind="ExternalInput")
    valc = nc.dram_tensor("valc", [nwv, PB, WIN], bf16, kind="ExternalInput")
    colsw = nc.dram_tensor("colsw", [nwv, PB, 16, 128], i16, kind="ExternalInput")
    rowsw = nc.dram_tensor("rowsw", [nwv, PB, 16, 128], i16, kind="ExternalInput")
    bndr = nc.dram_tensor("bndr", [nwv, PB, 16, 8], i16, kind="ExternalInput")
    bndc = nc.dram_tensor("bndc", [nwv, PB, 16, 8], i16, kind="ExternalInput")
    brow = nc.dram_tensor("brow", [nwv, PB, 128], f32, kind="ExternalInput")
    crow = nc.dram_tensor("crow", [nwv, PB, 128], f32, kind="ExternalInput")
    lamrow = nc.dram_tensor("lamrow", [nwv, PB, 128], f32, kind="ExternalInput")
    out = nc.dram_tensor("out", [4, PB], f32, kind="ExternalOutput")

    def rows16(ap, off=0):
        return ap.rearrange("(a b) f -> a b f", b=16)[:, off : off + 1, :]

    with tile.TileContext(nc) as tc, ExitStack() as ctx:
        const = ctx.enter_context(tc.tile_pool(name="const", bufs=1))
        zp = ctx.enter_context(tc.tile_pool(name="zp", bufs=1))
        gp = ctx.enter_context(tc.tile_pool(name="gp", bufs=1))
        st = ctx.enter_context(tc.tile_pool(name="st", bufs=2))
        pg = ctx.enter_context(tc.tile_pool(name="pg", bufs=2))
        wp = ctx.enter_context(tc.tile_pool(name="wp", bufs=1))

        # merged gather source: x_j on partition 16j, lam_j on 16j+8
        srcv = const.tile([128, N], f32, tag="srcv")
        nc.vector.memset(srcv[:], 0.0)
        for j in range(PB):
            nc.sync.dma_start(srcv[16 * j : 16 * j + 1, :], xs[j : j + 1, :])
            nc.sync.dma_start(
                srcv[16 * j + 8 : 16 * j + 9, :], lams[j : j + 1, :]
            )

        accP = const.tile([128, 1], f32, tag="accP")
        accC = const.tile([128, 1], f32, tag="accC")
        accS = const.tile([128, 1], f32, tag="accS")
        nc.vector.memset(accP[:], 0.0)
        nc.vector.memset(accC[:], 0.0)
        nc.vector.memset(accS[:], 0.0)

        # pre-zero rotating buffers whose unused rows must stay 0
        for tg in ("vr", "vc"):
            tz = zp.tile([128, GW], f32, tag=tg)
            nc.vector.memset(tz[:], 0.0)
        for tg in ("br", "cr", "lr"):
            tz = zp.tile([128, GB], f32, tag=tg)
            nc.vector.memset(tz[:], 0.0)
        for tg in ("dx", "dc"):
            tz = wp.tile([128, GB], f32, tag=tg)
            nc.vector.memset(tz[:], 0.0)

        for g in range(nwv // G):
            sl = slice(G * g, G * (g + 1))
            vr = zp.tile([128, GW], f32, tag="vr")
            nc.gpsimd.dma_start(rows16(vr[:]), valr[sl].rearrange("w j f -> j w f"))
            vc = zp.tile([128, GW], f32, tag="vc")
            nc.gpsimd.dma_start(
                rows16(vc[:], 8), valc[sl].rearrange("w j f -> j w f")
            )
            ixc = st.tile([128, G * 128], i16, tag="ixc")
            nc.sync.dma_start(
                ixc[:].rearrange("p (w s) -> p w s", w=G),
                colsw[sl].rearrange("w j q s -> (j q) w s"),
            )
            ixr = st.tile([128, G * 128], i16, tag="ixr")
            nc.sync.dma_start(
                ixr[:].rearrange("p (w s) -> p w s", w=G),
                rowsw[sl].rearrange("w j q s -> (j q) w s"),
            )
            ibr = st.tile([128, G * 8], i16, tag="ibr")
            nc.sync.dma_start(
                ibr[:].rearrange("p (w s) -> p w s", w=G),
                bndr[sl].rearrange("w j q s -> (j q) w s"),
            )
            ibc = st.tile([128, G * 8], i16, tag="ibc")
            nc.sync.dma_start(
                ibc[:].rearrange("p (w s) -> p w s", w=G),
                bndc[sl].rearrange("w j q s -> (j q) w s"),
            )
            br_t = zp.tile([128, GB], f32, tag="br")
            nc.sync.dma_start(rows16(br_t[:]), brow[sl].rearrange("w j f -> j w f"))
            cr_t = zp.tile([128, GB], f32, tag="cr")
            nc.sync.dma_start(
                rows16(cr_t[:], 8), crow[sl].rearrange("w j f -> j w f")
            )
            lr_t = zp.tile([128, GB], f32, tag="lr")
            nc.sync.dma_start(
                rows16(lr_t[:]), lamrow[sl].rearrange("w j f -> j w f")
            )

            gx = gp.tile([128, GW], f32, tag="gx")
            nc.gpsimd.ap_gather(gx[:], srcv[:], ixc[:], 128, N, 1, GW)
            gl = gp.tile([128, GW], f32, tag="gl")
            nc.gpsimd.ap_gather(gl[:], srcv[:], ixr[:], 128, M, 1, GW)

            nc.vector.tensor_tensor(gx[:], vr[:], gx[:], mybir.AluOpType.mult)
            nc.vector.tensor_tensor_scan(
                gx[:], gx[:], gx[:], 0.0, mybir.AluOpType.add,
                mybir.AluOpType.bypass,
            )
            nc.vector.tensor_tensor(gl[:], vc[:], gl[:], mybir.AluOpType.mult)
            nc.vector.tensor_tensor_scan(
                gl[:], gl[:], gl[:], 0.0, mybir.AluOpType.add,
                mybir.AluOpType.bypass,
            )

            Pgx = pg.tile([128, GB], f32, tag="pgx")
            nc.gpsimd.ap_gather(Pgx[:], gx[:], ibr[:], 128, GW, 1, GB)
            Pgl = pg.tile([128, GB], f32, tag="pgl")
            nc.gpsimd.ap_gather(Pgl[:], gl[:], ibc[:], 128, GW, 1, GB)

            dx = wp.tile([128, GB], f32, tag="dx")
            nc.vector.tensor_tensor(
                dx[:, 0 : GB - 1], Pgx[:, 1:GB], Pgx[:, 0 : GB - 1],
                mybir.AluOpType.subtract,
            )
            dab = wp.tile([128, GB], f32, tag="dab")
            nc.vector.tensor_tensor(dab[:], dx[:], br_t[:], mybir.AluOpType.subtract)
            rd = wp.tile([128, GB], f32, tag="rd")
            nc.vector.tensor_scalar(rd[:], dab[:], 0.0, None, mybir.AluOpType.max)
            rd2 = wp.tile([128, GB], f32, tag="rd2")
            nc.vector.tensor_tensor(rd2[:], rd[:], rd[:], mybir.AluOpType.mult)
            tpr = wp.tile([128, 1], f32, tag="tpr")
            nc.vector.tensor_reduce(
                tpr[:], rd2[:], mybir.AxisListType.X, mybir.AluOpType.add
            )
            nc.vector.tensor_tensor(accP[:], accP[:], tpr[:], mybir.AluOpType.add)
            ld = wp.tile([128, GB], f32, tag="ld")
            nc.vector.tensor_tensor(ld[:], lr_t[:], dab[:], mybir.AluOpType.mult)
            ld2 = wp.tile([128, GB], f32, tag="ld2")
            nc.vector.tensor_tensor(ld2[:], ld[:], ld[:], mybir.AluOpType.mult)
            tpc = wp.tile([128, 1], f32, tag="tpc")
            nc.vector.tensor_reduce(
                tpc[:], ld2[:], mybir.AxisListType.X, mybir.AluOpType.add
            )
            nc.vector.tensor_tensor(accC[:], accC[:], tpc[:], mybir.AluOpType.add)

            dc = wp.tile([128, GB], f32, tag="dc")
            nc.vector.tensor_tensor(
                dc[:, 0 : GB - 1], Pgl[:, 1:GB], Pgl[:, 0 : GB - 1],
                mybir.AluOpType.subtract,
            )
            stc = wp.tile([128, GB], f32, tag="stc")
            nc.vector.tensor_tensor(stc[:], dc[:], cr_t[:], mybir.AluOpType.add)
            st2 = wp.tile([128, GB], f32, tag="st2")
            nc.vector.tensor_tensor(st2[:], stc[:], stc[:], mybir.AluOpType.mult)
            tps = wp.tile([128, 1], f32, tag="tps")
            nc.vector.tensor_reduce(
                tps[:], st2[:], mybir.AxisListType.X, mybir.AluOpType.add
            )
            nc.vector.tensor_tensor(accS[:], accS[:], tps[:], mybir.AluOpType.add)

        # dual from lam rows {16j+8} of srcv (in place; gathers all done)
        nc.vector.tensor_scalar(srcv[:], srcv[:], 0.0, None, mybir.AluOpType.min)
        nc.vector.tensor_tensor(srcv[:], srcv[:], srcv[:], mybir.AluOpType.mult)
        accD = const.tile([128, 1], f32, tag="accD")
        nc.vector.tensor_reduce(
            accD[:], srcv[:], mybir.AxisListType.X, mybir.AluOpType.add
        )

        for k, (acc, off) in enumerate(
            ((accP, 0), (accC, 0), (accS, 8), (accD, 8))
        ):
            nc.sync.dma_start(out[k : k + 1, :], rows16(acc[:], off))

    nc.compile()
    return nc


def _pack_side(key_idx, other_idx, vals):
    order = np.argsort(key_idx, kind="stable")
    k_s = key_idx[order]
    o_s = other_idx[order].astype(np.int16)
    v_s = vals[order]
    counts = np.bincount(k_s, minlength=M)
    row_start = np.concatenate([[0], np.cumsum(counts)])
    spans = []
    base = 0
    while base < M:
        span = 0
        slots = 0
        while base + span < M and span < PW:
            c = counts[base + span]
            if slots + c > WIN - 1:
                break
            slots += c
            span += 1
        assert span > 0, "single row exceeds WIN-1"
        spans.append((base, span, slots))
        base += span
    return o_s, v_s, counts, row_start, spans


def _fill_side(nwv, packs):
    """Per-core side arrays: valw [nwv,PB,WIN] f32, gidx [nwv,PB,16,128] i16,
    bnd [nwv,PB,16,8] i16, prm [nwv,128,PB] int64 (-1 pad)."""
    valw = np.zeros((nwv, PB, WIN), np.float32)
    gidx = np.zeros((nwv, PB, 16, 128), np.int16)
    bnd = np.zeros((nwv, PB, 16, 8), np.int16)
    prm = np.full((nwv, 128, PB), -1, np.int64)
    for j, (o_s, v_s, counts, row_start, spans) in enumerate(packs):
        for w, (b0, span, slots) in enumerate(spans):
            lo, hi = row_start[b0], row_start[b0 + span]
            vwin = np.zeros(WIN, np.float32)
            owin = np.zeros(WIN, np.int16)
            vwin[1 : 1 + slots] = v_s[lo:hi]
            owin[1 : 1 + slots] = o_s[lo:hi]
            valw[w, j] = vwin
            gidx[w, j] = owin.reshape(128, 16).T
            cum = np.cumsum(counts[b0 : b0 + span])
            off = 2048 * (w % 4)
            bs = np.full(128, off, np.int64)
            for r in range(127):
                bs[r + 1] = off + cum[min(r, span - 1)]
            bnd[w, j] = bs.reshape(8, 16).T
            prm[w, :span, j] = np.arange(b0, b0 + span)
        for w in range(len(spans), nwv):
            bs = np.full(128, 2048 * (w % 4), np.int64)
            bnd[w, j] = bs.reshape(8, 16).T
    return valw, gidx, bnd, prm


def _apply_prm(vec, prm):
    """vec [PB, M] -> [nwv, PB, 128] permuted row-layout (0 where pad)."""
    nwv = prm.shape[0]
    outp = np.zeros((nwv, PB, 128), np.float32)
    for j in range(PB):
        m = prm[:, :, j]
        valid = m >= 0
        outp[:, j, :][valid] = vec[j][m[valid]]
    return outp


def kernel(x_hat, lam_hat, A_vals, A_rows, A_cols, b_pad, c_pad):
    global LAST_EXEC_NS, LAST_SINGLE_NS
    x = np.asarray(x_hat, dtype=np.float32).reshape(B, N)
    lam = np.asarray(lam_hat, dtype=np.float32).reshape(B, M)
    A_vals = np.ascontiguousarray(np.asarray(A_vals, dtype=np.float32))
    A_rows = np.ascontiguousarray(np.asarray(A_rows, dtype=np.int32))
    A_cols = np.ascontiguousarray(np.asarray(A_cols, dtype=np.int32))
    b_pad = np.ascontiguousarray(np.asarray(b_pad, dtype=np.float32))
    c_pad = np.ascontiguousarray(np.asarray(c_pad, dtype=np.float32))

    try:
        t0 = time.time()
        packs_r = [[None] * PB for _ in range(NCORES)]
        packs_c = [[None] * PB for _ in range(NCORES)]
        nwv = 0
        for i in range(B):
            ci, j = divmod(i, PB)
            pr = _pack_side(A_rows[i], A_cols[i], A_vals[i])
            pc = _pack_side(A_cols[i], A_rows[i], A_vals[i])
            packs_r[ci][j] = pr
            packs_c[ci][j] = pc
            nwv = max(nwv, len(pr[4]), len(pc[4]))
        nwv = (nwv + 3) // 4 * 4
        import ml_dtypes
        _bf = ml_dtypes.bfloat16
        in_maps = []
        for ci in range(NCORES):
            s = slice(PB * ci, PB * (ci + 1))
            valr, colsw, bndr, prm_r = _fill_side(nwv, packs_r[ci])
            valc, rowsw, bndc, prm_c = _fill_side(nwv, packs_c[ci])
            in_maps.append(
                {
                    "xs": np.ascontiguousarray(x[s]),
                    "lams": np.ascontiguousarray(lam[s]),
                    "valr": valr.astype(_bf),
                    "valc": valc.astype(_bf),
                    "colsw": colsw,
                    "rowsw": rowsw,
                    "bndr": bndr,
                    "bndc": bndc,
                    "brow": _apply_prm(b_pad[s], prm_r),
                    "lamrow": _apply_prm(lam[s], prm_r),
                    "crow": _apply_prm(c_pad[s], prm_c),
                }
            )
        print(f"[kernel] host prep {time.time()-t0:.1f}s, nwv={nwv}", flush=True)

        key = ("nc", nwv)
        if key not in _CACHED:
            _CACHED[key] = build_kernel(nwv)
        nc = _CACHED[key]

        results, per_exec_ns, single_ns = _run_timed(nc, in_maps, NCORES)
        LAST_EXEC_NS = per_exec_ns
        LAST_SINGLE_NS = single_ns
        print(
            f"[kernel] per-exec {per_exec_ns:.0f} ns, best {single_ns:.0f} ns",
            flush=True,
        )
    except Exception:
        import traceback

        traceback.print_exc()
        return _host_fallback(x, lam, A_vals, A_rows, A_cols, b_pad, c_pad)

    total = np.float64(0.0)
    for i in range(NCORES):
        v = np.asarray(results[i]["out"], dtype=np.float64).reshape(4, PB)
        for j in range(PB):
            total += (
                W_PRIMAL * v[0, j] / M
                + W_COMP * v[1, j] / M
                + W_STAT * v[2, j] / N
                + W_DUAL * v[3, j] / M
            )
    return np.float32(total / B)


def _host_fallback(x, lam, vals, rows, cols, b_pad, c_pad):
    print("[kernel] !!! HOST FALLBACK USED !!!", flush=True)
    tot = 0.0
    for i in range(B):
        Ax = np.bincount(rows[i], weights=(vals[i] * x[i][cols[i]]).astype(np.float64), minlength=M)
        ATl = np.bincount(cols[i], weights=(vals[i] * lam[i][rows[i]]).astype(np.float64), minlength=N)
        d = Ax - b_pad[i]
        tot += (W_PRIMAL * np.mean(np.maximum(d, 0.0) ** 2)
                + W_DUAL * np.mean(np.maximum(-lam[i], 0.0) ** 2)
                + W_STAT * np.mean((ATl + c_pad[i]) ** 2)
                + W_COMP * np.mean((lam[i] * d) ** 2))
    return np.float32(tot / B)


# revision 18
# speedup vs baseline: 1.0765x; 1.0671x over previous
"""KKT loss kernel v5 for Trainium2 (Bass/Tile), 8 NeuronCores.

Host sorts each problem's COO per side, packs complete scatter-rows into
2048-slot windows (slot 0 pad, span <= 127). Device per window: ap_gather
x[cols]/lam[rows] in sorted order (Q7 core j = problem j, row 16j),
s = v*g (in place), window-local inclusive prefix scan (DVE), boundary
ap_gather of the prefix at host-known row-end slots, adjacent diff =
per-row segment sums, loss terms accumulated against host-permuted
b/lam/c in row layout. No matmuls, no one-hots, no relayouts; emission is
stage-major with one-window lag on the boundary stage for pipeline slack.
"""

import os
import sys
import time

import numpy as np

sys.path.insert(0, "/opt/trn_rl_repo")

from contextlib import ExitStack

import concourse.bass as bass
import concourse.mybir as mybir
from concourse import bacc, tile

B, M, N, NNZ = 64, 8192, 8192, 262144
W_PRIMAL, W_DUAL, W_STAT, W_COMP = 0.1, 0.1, 0.6, 0.2

PB = 8
NCORES = 8
WIN = 2048
PW = 127

f32 = mybir.dt.float32
bf16 = mybir.dt.bfloat16
i16 = mybir.dt.int16

LAST_EXEC_NS = None
LAST_SINGLE_NS = None
_CACHED = {}


def _run_timed(nc, in_maps, n_cores, reps=5):
    """Execute the compiled Bass module on n_cores via PJRT; time repeats."""
    import time as _time

    import jax

    from jax.sharding import Mesh, PartitionSpec
    from jax.experimental.shard_map import shard_map

    from concourse import bass2jax, mybir as _mybir
    from concourse.bass2jax import _bass_exec_p, partition_id_tensor

    bass2jax.install_neuronx_cc_hook()

    partition_name = nc.partition_id_tensor.name if nc.partition_id_tensor else None
    in_names, out_names, out_avals, zero_outs = [], [], [], []
    for alloc in nc.m.functions[0].allocations:
        if not isinstance(alloc, _mybir.MemoryLocationSet):
            continue
        name = alloc.memorylocations[0].name
        if alloc.kind == "ExternalInput":
            if name != partition_name:
                in_names.append(name)
        elif alloc.kind == "ExternalOutput":
            shape = tuple(alloc.tensor_shape)
            dtype = _mybir.dt.np(alloc.dtype)
            out_names.append(name)
            out_avals.append(jax.core.ShapedArray(shape, dtype))
            zero_outs.append(np.zeros(shape, dtype))
    n_params = len(in_names)
    all_in_names = list(in_names) + list(out_names)
    if partition_name is not None:
        all_in_names.append(partition_name)

    def _body(*args):
        operands = list(args)
        if partition_name is not None:
            operands.append(partition_id_tensor())
        return tuple(
            _bass_exec_p.bind(
                *operands,
                out_avals=tuple(out_avals),
                in_names=tuple(all_in_names),
                out_names=tuple(out_names),
                lowering_input_output_aliases=(),
                sim_require_finite=False,
                sim_require_nnan=False,
                nc=nc,
            )
        )

    devices = jax.devices()[:n_cores]
    mesh = Mesh(np.asarray(devices), ("core",))
    n_outs = len(out_names)
    in_specs = (PartitionSpec("core"),) * (n_params + n_outs)
    out_specs = (PartitionSpec("core"),) * n_outs
    f1 = jax.jit(
        shard_map(_body, mesh=mesh, in_specs=in_specs, out_specs=out_specs,
                  check_rep=False),
        keep_unused=True,
    )

    if nc.dbg_addr is not None:
        dbg_zero = np.zeros((1, 2), np.uint32)
        in_maps = [{**m, nc.dbg_addr.name: dbg_zero} for m in in_maps]
    per_core = [[np.asarray(m[nm]) for nm in in_names] for m in in_maps]
    concat_in = [
        np.concatenate([per_core[c][i] for c in range(n_cores)], axis=0)
        for i in range(n_params)
    ]
    concat_zeros = [
        np.zeros((n_cores * z.shape[0], *z.shape[1:]), z.dtype) for z in zero_outs
    ]
    dev_in = [jax.device_put(a) for a in concat_in]
    dev_zeros = [jax.device_put(z) for z in concat_zeros]

    out1 = f1(*dev_in, *dev_zeros)
    jax.block_until_ready(out1)

    def _batch(k):
        best = None
        for _ in range(3):
            t0 = _time.perf_counter()
            os_ = [f1(*dev_in, *dev_zeros) for _ in range(k)]
            jax.block_until_ready(os_)
            dt = _time.perf_counter() - t0
            best = dt if best is None else min(best, dt)
        return best

    K = int(os.environ.get("KKT_TIME_K", "8"))
    tK = _batch(K)
    t2K = _batch(2 * K)
    per_exec_ns = (t2K - tK) / K * 1e9
    single_ns = tK / K * 1e9

    results = [
        {
            name: np.asarray(out1[i]).reshape(n_cores, *out_avals[i].shape)[c]
            for i, name in enumerate(out_names)
        }
        for c in range(n_cores)
    ]
    return results, per_exec_ns, single_ns




def _rows16(ap):
    """[128, F] AP -> [8, 1, F] view selecting partitions 16j."""
    return ap.rearrange("(a b) f -> a b f", b=16)[:, 0:1, :]


def build_kernel(nwv):
    G = 4
    GW = G * WIN          # 8192 slots per group
    GB = G * 128          # 512 boundary entries per group
    nc = bacc.Bacc(None, target_bir_lowering=False, debug=True)

    xs = nc.dram_tensor("xs", [PB, N], f32, kind="ExternalInput")
    lams = nc.dram_tensor("lams", [PB, M], f32, kind="ExternalInput")
    valr = nc.dram_tensor("valr", [nwv, PB, WIN], bf16, kind="ExternalInput")
    valc = nc.dram_tensor("valc", [nwv, PB, WIN], bf16, kind="ExternalInput")
    colsw = nc.dram_tensor("colsw", [nwv, PB, 16, 128], i16, kind="ExternalInput")
    rowsw = nc.dram_tensor("rowsw", [nwv, PB, 16, 128], i16, kind="ExternalInput")
    bndr = nc.dram_tensor("bndr", [nwv, PB, 16, 8], i16, kind="ExternalInput")
    bndc = nc.dram_tensor("bndc", [nwv, PB, 16, 8], i16, kind="ExternalInput")
    brow = nc.dram_tensor("brow", [nwv, PB, 128], f32, kind="ExternalInput")
    crow = nc.dram_tensor("crow", [nwv, PB, 128], f32, kind="ExternalInput")
    lamrow = nc.dram_tensor("lamrow", [nwv, PB, 128], f32, kind="ExternalInput")
    out = nc.dram_tensor("out", [4, PB], f32, kind="ExternalOutput")

    def rows16(ap, off=0):
        return ap.rearrange("(a b) f -> a b f", b=16)[:, off : off + 1, :]

    with tile.TileContext(nc) as tc, ExitStack() as ctx:
        const = ctx.enter_context(tc.tile_pool(name="const", bufs=1))
        zp = ctx.enter_context(tc.tile_pool(name="zp", bufs=1))
        gp = ctx.enter_context(tc.tile_pool(name="gp", bufs=1))
        st = ctx.enter_context(tc.tile_pool(name="st", bufs=2))
        pg = ctx.enter_context(tc.tile_pool(name="pg", bufs=2))
        wp = ctx.enter_context(tc.tile_pool(name="wp", bufs=1))

        # merged gather source: x_j on partition 16j, lam_j on 16j+8
        srcv = const.tile([128, N], f32, tag="srcv")
        nc.vector.memset(srcv[:], 0.0)
        for j in range(PB):
            nc.sync.dma_start(srcv[16 * j : 16 * j + 1, :], xs[j : j + 1, :])
            nc.sync.dma_start(
                srcv[16 * j + 8 : 16 * j + 9, :], lams[j : j + 1, :]
            )

        accP = const.tile([128, 1], f32, tag="accP")
        accC = const.tile([128, 1], f32, tag="accC")
        accS = const.tile([128, 1], f32, tag="accS")
        nc.vector.memset(accP[:], 0.0)
        nc.vector.memset(accC[:], 0.0)
        nc.vector.memset(accS[:], 0.0)

        vf = const.tile([128, GW], f32, tag="vf")
        # pre-zero rotating buffers whose unused rows must stay 0
        for tg in ("vr", "vc"):
            tz = zp.tile([128, GW], bf16, tag=tg)
            nc.vector.memset(tz[:], 0.0)
        for tg in ("br", "cr", "lr"):
            tz = zp.tile([128, GB], f32, tag=tg)
            nc.vector.memset(tz[:], 0.0)
        for tg in ("dx", "dc"):
            tz = wp.tile([128, GB], f32, tag=tg)
            nc.vector.memset(tz[:], 0.0)

        for g in range(nwv // G):
            sl = slice(G * g, G * (g + 1))
            vr = zp.tile([128, GW], bf16, tag="vr")
            nc.sync.dma_start(rows16(vr[:]), valr[sl].rearrange("w j f -> j w f"))
            vc = zp.tile([128, GW], bf16, tag="vc")
            nc.sync.dma_start(
                rows16(vc[:], 8), valc[sl].rearrange("w j f -> j w f")
            )
            ixc = st.tile([128, G * 128], i16, tag="ixc")
            nc.sync.dma_start(
                ixc[:].rearrange("p (w s) -> p w s", w=G),
                colsw[sl].rearrange("w j q s -> (j q) w s"),
            )
            ixr = st.tile([128, G * 128], i16, tag="ixr")
            nc.sync.dma_start(
                ixr[:].rearrange("p (w s) -> p w s", w=G),
                rowsw[sl].rearrange("w j q s -> (j q) w s"),
            )
            ibr = st.tile([128, G * 8], i16, tag="ibr")
            nc.sync.dma_start(
                ibr[:].rearrange("p (w s) -> p w s", w=G),
                bndr[sl].rearrange("w j q s -> (j q) w s"),
            )
            ibc = st.tile([128, G * 8], i16, tag="ibc")
            nc.sync.dma_start(
                ibc[:].rearrange("p (w s) -> p w s", w=G),
                bndc[sl].rearrange("w j q s -> (j q) w s"),
            )
            br_t = zp.tile([128, GB], f32, tag="br")
            nc.sync.dma_start(rows16(br_t[:]), brow[sl].rearrange("w j f -> j w f"))
            cr_t = zp.tile([128, GB], f32, tag="cr")
            nc.sync.dma_start(
                rows16(cr_t[:], 8), crow[sl].rearrange("w j f -> j w f")
            )
            lr_t = zp.tile([128, GB], f32, tag="lr")
            nc.sync.dma_start(
                rows16(lr_t[:]), lamrow[sl].rearrange("w j f -> j w f")
            )

            gx = gp.tile([128, GW], f32, tag="gx")
            nc.gpsimd.ap_gather(gx[:], srcv[:], ixc[:], 128, N, 1, GW)
            gl = gp.tile([128, GW], f32, tag="gl")
            nc.gpsimd.ap_gather(gl[:], srcv[:], ixr[:], 128, M, 1, GW)

            nc.vector.tensor_copy(vf[:], vr[:])
            nc.vector.tensor_tensor(gx[:], vf[:], gx[:], mybir.AluOpType.mult)
            nc.vector.tensor_tensor_scan(
                gx[:], gx[:], gx[:], 0.0, mybir.AluOpType.add,
                mybir.AluOpType.bypass,
            )
            nc.vector.tensor_copy(vf[:], vc[:])
            nc.vector.tensor_tensor(gl[:], vf[:], gl[:], mybir.AluOpType.mult)
            nc.vector.tensor_tensor_scan(
                gl[:], gl[:], gl[:], 0.0, mybir.AluOpType.add,
                mybir.AluOpType.bypass,
            )

            Pgx = pg.tile([128, GB], f32, tag="pgx")
            nc.gpsimd.ap_gather(Pgx[:], gx[:], ibr[:], 128, GW, 1, GB)
            Pgl = pg.tile([128, GB], f32, tag="pgl")
            nc.gpsimd.ap_gather(Pgl[:], gl[:], ibc[:], 128, GW, 1, GB)

            dx = wp.tile([128, GB], f32, tag="dx")
            nc.vector.tensor_tensor(
                dx[:, 0 : GB - 1], Pgx[:, 1:GB], Pgx[:, 0 : GB - 1],
                mybir.AluOpType.subtract,
            )
            dab = wp.tile([128, GB], f32, tag="dab")
            nc.vector.tensor_tensor(dab[:], dx[:], br_t[:], mybir.AluOpType.subtract)
            rd = wp.tile([128, GB], f32, tag="rd")
            nc.vector.tensor_scalar(rd[:], dab[:], 0.0, None, mybir.AluOpType.max)
            rd2 = wp.tile([128, GB], f32, tag="rd2")
            nc.vector.tensor_tensor(rd2[:], rd[:], rd[:], mybir.AluOpType.mult)
            tpr = wp.tile([128, 1], f32, tag="tpr")
            nc.vector.tensor_reduce(
                tpr[:], rd2[:], mybir.AxisListType.X, mybir.AluOpType.add
            )
            nc.vector.tensor_tensor(accP[:], accP[:], tpr[:], mybir.AluOpType.add)
            ld = wp.tile([128, GB], f32, tag="ld")
            nc.vector.tensor_tensor(ld[:], lr_t[:], dab[:], mybir.AluOpType.mult)
            ld2 = wp.tile([128, GB], f32, tag="ld2")
            nc.vector.tensor_tensor(ld2[:], ld[:], ld[:], mybir.AluOpType.mult)
            tpc = wp.tile([128, 1], f32, tag="tpc")
            nc.vector.tensor_reduce(
                tpc[:], ld2[:], mybir.AxisListType.X, mybir.AluOpType.add
            )
            nc.vector.tensor_tensor(accC[:], accC[:], tpc[:], mybir.AluOpType.add)

            dc = wp.tile([128, GB], f32, tag="dc")
            nc.vector.tensor_tensor(
                dc[:, 0 : GB - 1], Pgl[:, 1:GB], Pgl[:, 0 : GB - 1],
                mybir.AluOpType.subtract,
            )
            stc = wp.tile([128, GB], f32, tag="stc")
            nc.vector.tensor_tensor(stc[:], dc[:], cr_t[:], mybir.AluOpType.add)
            st2 = wp.tile([128, GB], f32, tag="st2")
            nc.vector.tensor_tensor(st2[:], stc[:], stc[:], mybir.AluOpType.mult)
            tps = wp.tile([128, 1], f32, tag="tps")
            nc.vector.tensor_reduce(
                tps[:], st2[:], mybir.AxisListType.X, mybir.AluOpType.add
            )
            nc.vector.tensor_tensor(accS[:], accS[:], tps[:], mybir.AluOpType.add)

        # dual from lam rows {16j+8} of srcv (in place; gathers all done)
        nc.vector.tensor_scalar(srcv[:], srcv[:], 0.0, None, mybir.AluOpType.min)
        nc.vector.tensor_tensor(srcv[:], srcv[:], srcv[:], mybir.AluOpType.mult)
        accD = const.tile([128, 1], f32, tag="accD")
        nc.vector.tensor_reduce(
            accD[:], srcv[:], mybir.AxisListType.X, mybir.AluOpType.add
        )

        for k, (acc, off) in enumerate(
            ((accP, 0), (accC, 0), (accS, 8), (accD, 8))
        ):
            nc.sync.dma_start(out[k : k + 1, :], rows16(acc[:], off))

    nc.compile()
    return nc


def _pack_side(key_idx, other_idx, vals):
    order = np.argsort(key_idx, kind="stable")
    k_s = key_idx[order]
    o_s = other_idx[order].astype(np.int16)
    v_s = vals[order]
    counts = np.bincount(k_s, minlength=M)
    row_start = np.concatenate([[0], np.cumsum(counts)])
    spans = []
    base = 0
    while base < M:
        span = 0
        slots = 0
        while base + span < M and span < PW:
            c = counts[base + span]
            if slots + c > WIN - 1:
                break
            slots += c
            span += 1
        assert span > 0, "single row exceeds WIN-1"
        spans.append((base, span, slots))
        base += span
    return o_s, v_s, counts, row_start, spans


def _fill_side(nwv, packs):
    """Per-core side arrays: valw [nwv,PB,WIN] f32, gidx [nwv,PB,16,128] i16,
    bnd [nwv,PB,16,8] i16, prm [nwv,128,PB] int64 (-1 pad)."""
    valw = np.zeros((nwv, PB, WIN), np.float32)
    gidx = np.zeros((nwv, PB, 16, 128), np.int16)
    bnd = np.zeros((nwv, PB, 16, 8), np.int16)
    prm = np.full((nwv, 128, PB), -1, np.int64)
    for j, (o_s, v_s, counts, row_start, spans) in enumerate(packs):
        for w, (b0, span, slots) in enumerate(spans):
            lo, hi = row_start[b0], row_start[b0 + span]
            vwin = np.zeros(WIN, np.float32)
            owin = np.zeros(WIN, np.int16)
            vwin[1 : 1 + slots] = v_s[lo:hi]
            owin[1 : 1 + slots] = o_s[lo:hi]
            valw[w, j] = vwin
            gidx[w, j] = owin.reshape(128, 16).T
            cum = np.cumsum(counts[b0 : b0 + span])
            off = 2048 * (w % 4)
            bs = np.full(128, off, np.int64)
            for r in range(127):
                bs[r + 1] = off + cum[min(r, span - 1)]
            bnd[w, j] = bs.reshape(8, 16).T
            prm[w, :span, j] = np.arange(b0, b0 + span)
        for w in range(len(spans), nwv):
            bs = np.full(128, 2048 * (w % 4), np.int64)
            bnd[w, j] = bs.reshape(8, 16).T
    return valw, gidx, bnd, prm


def _apply_prm(vec, prm):
    """vec [PB, M] -> [nwv, PB, 128] permuted row-layout (0 where pad)."""
    nwv = prm.shape[0]
    outp = np.zeros((nwv, PB, 128), np.float32)
    for j in range(PB):
        m = prm[:, :, j]
        valid = m >= 0
        outp[:, j, :][valid] = vec[j][m[valid]]
    return outp


def kernel(x_hat, lam_hat, A_vals, A_rows, A_cols, b_pad, c_pad):
    global LAST_EXEC_NS, LAST_SINGLE_NS
    x = np.asarray(x_hat, dtype=np.float32).reshape(B, N)
    lam = np.asarray(lam_hat, dtype=np.float32).reshape(B, M)
    A_vals = np.ascontiguousarray(np.asarray(A_vals, dtype=np.float32))
    A_rows = np.ascontiguousarray(np.asarray(A_rows, dtype=np.int32))
    A_cols = np.ascontiguousarray(np.asarray(A_cols, dtype=np.int32))
    b_pad = np.ascontiguousarray(np.asarray(b_pad, dtype=np.float32))
    c_pad = np.ascontiguousarray(np.asarray(c_pad, dtype=np.float32))

    try:
        t0 = time.time()
        packs_r = [[None] * PB for _ in range(NCORES)]
        packs_c = [[None] * PB for _ in range(NCORES)]
        nwv = 0
        for i in range(B):
            ci, j = divmod(i, PB)
            pr = _pack_side(A_rows[i], A_cols[i], A_vals[i])
            pc = _pack_side(A_cols[i], A_rows[i], A_vals[i])
            packs_r[ci][j] = pr
            packs_c[ci][j] = pc
            nwv = max(nwv, len(pr[4]), len(pc[4]))
        nwv = (nwv + 3) // 4 * 4
        import ml_dtypes
        _bf = ml_dtypes.bfloat16
        in_maps = []
        for ci in range(NCORES):
            s = slice(PB * ci, PB * (ci + 1))
            valr, colsw, bndr, prm_r = _fill_side(nwv, packs_r[ci])
            valc, rowsw, bndc, prm_c = _fill_side(nwv, packs_c[ci])
            in_maps.append(
                {
                    "xs": np.ascontiguousarray(x[s]),
                    "lams": np.ascontiguousarray(lam[s]),
                    "valr": valr.astype(_bf),
                    "valc": valc.astype(_bf),
                    "colsw": colsw,
                    "rowsw": rowsw,
                    "bndr": bndr,
                    "bndc": bndc,
                    "brow": _apply_prm(b_pad[s], prm_r),
                    "lamrow": _apply_prm(lam[s], prm_r),
                    "crow": _apply_prm(c_pad[s], prm_c),
                }
            )
        print(f"[kernel] host prep {time.time()-t0:.1f}s, nwv={nwv}", flush=True)

        key = ("nc", nwv)
        if key not in _CACHED:
            _CACHED[key] = build_kernel(nwv)
        nc = _CACHED[key]

        results, per_exec_ns, single_ns = _run_timed(nc, in_maps, NCORES)
        LAST_EXEC_NS = per_exec_ns
        LAST_SINGLE_NS = single_ns
        print(
            f"[kernel] per-exec {per_exec_ns:.0f} ns, best {single_ns:.0f} ns",
            flush=True,
        )
    except Exception:
        import traceback

        traceback.print_exc()
        return _host_fallback(x, lam, A_vals, A_rows, A_cols, b_pad, c_pad)

    total = np.float64(0.0)
    for i in range(NCORES):
        v = np.asarray(results[i]["out"], dtype=np.float64).reshape(4, PB)
        for j in range(PB):
            total += (
                W_PRIMAL * v[0, j] / M
                + W_COMP * v[1, j] / M
                + W_STAT * v[2, j] / N
                + W_DUAL * v[3, j] / M
            )
    return np.float32(total / B)


def _host_fallback(x, lam, vals, rows, cols, b_pad, c_pad):
    print("[kernel] !!! HOST FALLBACK USED !!!", flush=True)
    tot = 0.0
    for i in range(B):
        Ax = np.bincount(rows[i], weights=(vals[i] * x[i][cols[i]]).astype(np.float64), minlength=M)
        ATl = np.bincount(cols[i], weights=(vals[i] * lam[i][rows[i]]).astype(np.float64), minlength=N)
        d = Ax - b_pad[i]
        tot += (W_PRIMAL * np.mean(np.maximum(d, 0.0) ** 2)
                + W_DUAL * np.mean(np.maximum(-lam[i], 0.0) ** 2)
                + W_STAT * np.mean((ATl + c_pad[i]) ** 2)
                + W_COMP * np.mean((lam[i] * d) ** 2))
    return np.float32(tot / B)
